# revision 28
# baseline (speedup 1.0000x reference)
"""Trainium2 Bass kernel for nn_LossFunc_13752485282042 (chamfer + class + KL + histogram loss).

Contract: kernel(**inputs) takes FULL unsharded numpy inputs (B=256) and returns the
full [256] f32 loss vector. Internally shards batch across 8 NeuronCores (pure data
parallel, 32 samples/core) and runs one SPMD Bass/Tile kernel.

Algorithm per sample (N=M=512 points, K=4 kine dims, D=9 classes):
  f1[n,m] = sum_k x[n,k]*y[m,k] - 0.5*|y[m]|^2        (row-tiled KC=27 bf16-split matmuls)
  min_m d = 2*relu(0.5|x_n|^2 - max_m f1); row max via DVE tensor_reduce(max).
  Class gather is realized WITHOUT index extraction: a {0,1} winner mask
  mask1[n,m] = (f1[n,m] == rowmax1[n]) (DVE is_equal, or ACT Sign with a host-side
  colsum correction), then C1[m,d] = sum_n mask1[n,m]*ci[n,d] via 16 small matmuls,
  and term1 = <C1, cp> via one DVE scalar_tensor_tensor with accum. Same transposed
  for the pred->input direction. Histogram/classnum via equality masks + selector
  matmuls; KL via ACT exp-accumulate. No GPSIMD gathers, no index relayout DMAs.
"""

import numpy as np

import concourse.bass as bass
import concourse.bacc as bacc
import concourse.mybir as mybir
from concourse.tile import TileContext
from concourse.bass_utils import run_bass_kernel_spmd

F32 = mybir.dt.float32
BF16 = mybir.dt.bfloat16
AX = mybir.AxisListType
OP = mybir.AluOpType
ACT = mybir.ActivationFunctionType

B, N, K, D, L = 256, 512, 4, 9, 32
KC = 27                   # matmul contraction rows: 6 bf16-split products x 4 dims + 3 |y|^2 rows
NCORES = 8
BS = B // NCORES          # 32 samples per core
NCH = N // 128            # 4 partition chunks per sample

# Winner-mask engine: False -> DVE is_equal ({0,1} mask, no correction);
# True -> ACT Sign(rowmax - f1) ({0 winner, +1 loser} mask, host colsum correction).
import os as _os
MASK_ON_ACT = _os.environ.get("MASK_ON_ACT", "1") == "1"
OPS_BUFS = int(_os.environ.get("OPS_BUFS", "6"))
MASK_BUFS = int(_os.environ.get("MASK_BUFS", "8"))
PS_BUFS = int(_os.environ.get("PS_BUFS", "3"))
C_BUFS = int(_os.environ.get("C_BUFS", "2"))
# move one winner-mask chunk per sample from ACT to DVE on samples where
# (s % MASK_MIX_MOD) == 1; 0 disables
MASK_MIX_MOD = int(_os.environ.get("MASK_MIX_MOD", "0"))

TRACE = False             # set by test.py to collect a profile
LAST_RESULT = None


def _build_core_program():
    nc = bacc.Bacc()

    ops1 = nc.declare_dram_parameter("ops1", [BS, NCH, 32, 128 + N], BF16, isOutput=False)
    ops2 = nc.declare_dram_parameter("ops2", [BS, NCH, 32, 128 + N], BF16, isOutput=False)
    cib = nc.declare_dram_parameter("cib", [128, BS, NCH, D], BF16, isOutput=False)
    cpb = nc.declare_dram_parameter("cpb", [128, BS, NCH, D], BF16, isOutput=False)
    dotp = nc.declare_dram_parameter("dotp", [128, BS, 2 * NCH * D], F32, isOutput=False)
    x2h1 = nc.declare_dram_parameter("x2h1", [128, BS * NCH], F32, isOutput=False)
    x2h2 = nc.declare_dram_parameter("x2h2", [128, BS * NCH], F32, isOutput=False)
    hci = nc.declare_dram_parameter("hci", [128, 128 * D], F32, isOutput=False)
    hcp = nc.declare_dram_parameter("hcp", [128, 128 * D], F32, isOutput=False)
    mu = nc.declare_dram_parameter("mu", [BS, L], F32, isOutput=False)
    lv = nc.declare_dram_parameter("lv", [BS, L], F32, isOutput=False)
    corr = nc.declare_dram_parameter("corr", [1, BS], F32, isOutput=False)
    c_eye = nc.declare_dram_parameter("c_eye", [BS, BS], F32, isOutput=False)
    c_ones = nc.declare_dram_parameter("c_ones", [128, 1], F32, isOutput=False)
    c_selp = nc.declare_dram_parameter("c_selp", [128, BS], F32, isOutput=False)
    c_seln = nc.declare_dram_parameter("c_seln", [128, BS], F32, isOutput=False)
    c_wrep = nc.declare_dram_parameter("c_wrep", [BS, D], F32, isOutput=False)
    out = nc.declare_dram_parameter("out", [BS], F32, isOutput=True)

    with TileContext(nc) as tc:
        _emit(nc, tc, ops1, ops2, cib, cpb, dotp, x2h1, x2h2, hci, hcp,
              mu, lv, corr, c_eye, c_ones, c_selp, c_seln, c_wrep, out)
    nc.finalize()
    return nc


def _host_consts():
    selp = np.zeros((128, BS), np.float32)
    for s in range(BS):
        for c in range(4):
            selp[32 * c + s, s] = 1.0
    wrep = np.ones((BS, D), np.float32)
    wrep[:, 0] = 2.0
    wrep[:, D - 1] = 100.0
    return {
        "c_eye": np.eye(BS, dtype=np.float32),
        "c_ones": np.ones((128, 1), np.float32),
        "c_selp": selp,
        "c_seln": -selp,
        "c_wrep": wrep,
    }


CONSTS = _host_consts()


def _emit(nc, tc, ops1d, ops2d, cibd, cpbd, dotpd, x2h1d, x2h2d, hcid, hcpd,
          mud, lvd, corrd, c_eye, c_ones, c_selp, c_seln, c_wrep, out):
    from contextlib import ExitStack

    ctx = ExitStack()
    with ctx:
        singles = ctx.enter_context(tc.tile_pool(name="singles", bufs=1))
        opsp = ctx.enter_context(tc.tile_pool(name="opsp", bufs=OPS_BUFS))
        maskp = ctx.enter_context(tc.tile_pool(name="maskp", bufs=MASK_BUFS))

        # ---------- tiles needed by the main loop ----------
        # class-data loads are quartered and interleaved with the first OPS
        # loads (emitted inside the loop) so sample 0's pipeline starts early
        CIB = singles.tile([128, BS, NCH, D], BF16, tag="CIB")
        CPB = singles.tile([128, BS, NCH, D], BF16, tag="CPB")
        DOTP = singles.tile([128, BS, 2 * NCH * D], F32, tag="DOTP")

        def _cls_quarter(q):
            sl = slice(8 * q, 8 * q + 8)
            nc.sync.dma_start(out=CIB[:, sl], in_=cibd[:, sl])
            nc.sync.dma_start(out=CPB[:, sl], in_=cpbd[:, sl])
            nc.sync.dma_start(out=DOTP[:, sl], in_=dotpd[:, sl])

        RMAX1 = singles.tile([128, BS * NCH], F32, tag="RMAX1")
        RMAX2 = singles.tile([128, BS * NCH], F32, tag="RMAX2")
        TD = singles.tile([128, BS], F32, tag="TD")
        DOTSCR = singles.tile([128, 2 * NCH * D], F32, tag="DOTSCR")

        ONES128 = singles.tile([128, 1], F32, tag="ONES128")
        SELP = singles.tile([128, BS], F32, tag="SELP")
        SELN = singles.tile([128, BS], F32, tag="SELN")
        WREP = singles.tile([BS, D], F32, tag="WREP")
        CORR = singles.tile([1, BS], F32, tag="CORR")
        EYE = singles.tile([BS, BS], F32, tag="EYE")
        X2H1 = singles.tile([128, BS * NCH], F32, tag="X2H1")
        X2H2 = singles.tile([128, BS * NCH], F32, tag="X2H2")
        HCI = singles.tile([128, 128 * D], F32, tag="HCI")
        HCP = singles.tile([128, 128 * D], F32, tag="HCP")
        KLCN = singles.tile([BS, 2], F32, tag="KLCN")

        def _emit_side_loads_a():
            nc.sync.dma_start(out=ONES128, in_=c_ones[:])
            nc.sync.dma_start(out=SELP, in_=c_selp[:])
            nc.sync.dma_start(out=SELN, in_=c_seln[:])
            nc.sync.dma_start(out=WREP, in_=c_wrep[:])
            nc.sync.dma_start(out=CORR, in_=corrd[:])
            nc.sync.dma_start(out=EYE, in_=c_eye[:])
            nc.sync.dma_start(out=X2H1, in_=x2h1d[:])
            nc.sync.dma_start(out=X2H2, in_=x2h2d[:])

        def _emit_side_loads_b():
            nc.sync.dma_start(out=HCI, in_=hcid[:])
            nc.sync.dma_start(out=HCP, in_=hcpd[:])

        def _emit_side_loads():
            # KL -> KLCN[:, 0]
            smu = singles.tile([BS, L], F32, tag="smu")
            slv = singles.tile([BS, L], F32, tag="slv")
            nc.sync.dma_start(out=smu, in_=mud[:])
            nc.sync.dma_start(out=slv, in_=lvd[:])
            sexp = singles.tile([BS, L], F32, tag="sexp")
            sumexp = singles.tile([BS, 1], F32, tag="sumexp")
            nc.scalar.activation(sexp, slv, ACT.Exp, accum_out=sumexp)
            smu2 = singles.tile([BS, L], F32, tag="smu2")
            summu2 = singles.tile([BS, 1], F32, tag="summu2")
            nc.vector.scalar_tensor_tensor(
                out=smu2, in0=smu, scalar=1.0, in1=smu,
                op0=OP.mult, op1=OP.mult, accum_out=summu2,
            )
            sumlv = singles.tile([BS, 1], F32, tag="sumlv")
            nc.vector.reduce_sum(sumlv, slv, axis=AX.X)
            klt = KLCN[:, 0:1]
            nc.vector.tensor_sub(klt, sumlv, summu2)
            nc.vector.tensor_sub(klt, klt, sumexp)
            nc.vector.tensor_scalar(out=klt, in0=klt, scalar1=float(L), scalar2=None, op0=OP.add)
            nc.vector.tensor_scalar(out=klt, in0=klt, scalar1=-0.5, scalar2=None, op0=OP.mult)

        def _emit_hist(psC):
            # histogram counts (equality mask on the otherwise-idle GpSimd),
            # then classnum -> KLCN[:, 1]
            cnts = []
            for name, h in (("i", HCI), ("p", HCP)):
                rmx = singles.tile([128, 128], F32, tag=f"rmx{name}")
                nc.vector.reduce_max(rmx, h.rearrange("p (n d) -> p n d", d=D), axis=AX.X)
                oh = singles.tile([128, 128 * D], F32, tag=f"oh{name}")
                nc.vector.tensor_tensor(
                    out=oh.rearrange("p (d n) -> p n d", d=D),
                    in0=h.rearrange("p (n d) -> p n d", d=D),
                    in1=rmx.to_broadcast([128, 128, D]),
                    op=OP.is_equal,
                )
                cnt = singles.tile([128, D], F32, tag=f"cnt{name}")
                nc.vector.reduce_sum(cnt, oh.rearrange("p (d n) -> p d n", d=D), axis=AX.X)
                cnts.append(cnt)
            psh = psC.tile([128, 2 * NCH * D], F32, tag="C")
            nc.tensor.matmul(psh[0:BS, 0:D], SELP, cnts[0], start=True, stop=False)
            nc.tensor.matmul(psh[0:BS, 0:D], SELN, cnts[1], start=False, stop=True)
            habs = singles.tile([BS, D], F32, tag="habs")
            nc.scalar.activation(habs, psh[0:BS, 0:D], ACT.Abs)
            hw_ = singles.tile([BS, D], F32, tag="hw_")
            nc.vector.tensor_mul(hw_, habs, WREP)
            nc.vector.reduce_sum(KLCN[:, 1:2], hw_, axis=AX.X)

        # ---------- main per-sample loop ----------
        with tc.tile_pool(name="psPS", bufs=PS_BUFS, space="PSUM") as psPS, \
             tc.tile_pool(name="psC", bufs=C_BUFS, space="PSUM") as psC:
            for s in range(BS):
                C = psC.tile([128, 2 * NCH * D], F32, tag="C")
                for di, (opsd, RMAXd, CLSR) in enumerate(
                    [(ops1d, RMAX1, CIB), (ops2d, RMAX2, CPB)]
                ):
                    OPS = opsp.tile([128, 128 + N], BF16, tag="OPS")
                    nc.sync.dma_start(
                        out=OPS, in_=opsd[s].rearrange("c r x -> (c r) x")
                    )
                    if s == 0 and di == 0:
                        _cls_quarter(0)
                    elif s == 1 and di == 0:
                        _cls_quarter(1)
                    elif s == 2 and di == 0:
                        _cls_quarter(2)
                        _cls_quarter(3)
                    masks = []
                    for h in range(2):
                        PS = psPS.tile([128, 2, N], F32, tag="PS")
                        for j in range(2):
                            c = 2 * h + j
                            base = 32 * c
                            nc.tensor.matmul(
                                PS[:, j],
                                OPS[base : base + KC, 0:128],
                                OPS[base : base + KC, 128:],
                                start=True, stop=True,
                                tile_position=(base, 0),
                            )
                        rms = RMAXd[:, NCH * s + 2 * h : NCH * s + 2 * h + 2]
                        nc.vector.reduce_max(rms, PS, axis=AX.X)
                        for j in range(2):
                            c = 2 * h + j
                            M = maskp.tile([128, N], BF16, tag="M")
                            on_dve = (
                                MASK_MIX_MOD
                                and s % MASK_MIX_MOD == 1
                                and di == 1 and h == 1 and j == 1
                            )
                            if MASK_ON_ACT and not on_dve:
                                # {0 at winner, +1 at losers}
                                nc.scalar.activation(
                                    M, PS[:, j], ACT.Sign,
                                    bias=rms[:, j : j + 1], scale=-1.0,
                                )
                            elif MASK_ON_ACT:
                                # DVE equivalent of the Sign mask: rmax - PS,
                                # clamped into {0 loser, ... } -- use is_gt:
                                # (rmax > PS) -> 1 at losers, 0 at winner
                                nc.vector.tensor_tensor(
                                    out=M,
                                    in0=rms[:, j : j + 1].to_broadcast([128, N]),
                                    in1=PS[:, j],
                                    op=OP.is_gt,
                                )
                            else:
                                # {1 at winner, 0 at losers}
                                nc.vector.tensor_tensor(
                                    out=M, in0=PS[:, j],
                                    in1=rms[:, j : j + 1].to_broadcast([128, N]),
                                    op=OP.is_equal,
                                )
                            masks.append(M)
                    # PSUM zeroing is bank-granular at group start, so each
                    # (di, mc) accumulation group must run back-to-back.
                    for mc in range(NCH):
                        for c in range(NCH):
                            nc.tensor.matmul(
                                C[:, D * (NCH * di + mc) : D * (NCH * di + mc) + D],
                                masks[c][:, 128 * mc : 128 * mc + 128],
                                CLSR[:, s, c, :],
                                start=(c == 0), stop=(c == NCH - 1),
                            )
                nc.vector.scalar_tensor_tensor(
                    out=DOTSCR, in0=C, scalar=1.0, in1=DOTP[:, s],
                    op0=OP.mult, op1=OP.mult,
                    accum_out=TD[:, s : s + 1],
                )
                if s == 6:
                    _emit_side_loads_a()
                elif s == 12:
                    _emit_side_loads_b()
                elif s == 16:
                    _emit_side_loads()
                elif s == 26:
                    _emit_hist(psC)

        # ---------- final assembly ----------
        with tc.tile_pool(name="psend", bufs=1, space="PSUM") as psend:
            # chamfer: relu(0.5|x|^2 - rowmax), summed over points
            chrows = []
            for X2H, RMAX, nm in ((X2H1, RMAX1, "1"), (X2H2, RMAX2, "2")):
                T = singles.tile([128, BS * NCH], F32, tag=f"T{nm}")
                nc.vector.scalar_tensor_tensor(
                    out=T, in0=RMAX, scalar=-1.0, in1=X2H,
                    op0=OP.mult, op1=OP.add,
                )
                R = singles.tile([128, BS * NCH], F32, tag=f"R{nm}")
                nc.vector.tensor_scalar(out=R, in0=T, scalar1=0.0, scalar2=None, op0=OP.max)
                pse = psend.tile([1, BS * NCH], F32, tag=f"pse{nm}")
                nc.tensor.matmul(pse, ONES128, R, start=True, stop=True)
                ch = singles.tile([1, BS], F32, tag=f"ch{nm}")
                nc.vector.reduce_sum(ch, pse.rearrange("o (s c) -> o s c", c=NCH), axis=AX.X)
                chrows.append(ch)

            # class-gather dot: partition-sum of TD
            pstd = psend.tile([1, BS], F32, tag="pstd")
            nc.tensor.matmul(pstd, ONES128, TD, start=True, stop=True)

            # [BS,1] -> [1,BS] via PE transpose (PSUM reads must start at partition 0)
            psT1 = psend.tile([1, BS], F32, tag="psT1")
            nc.tensor.transpose(psT1, KLCN[:, 0:1], EYE)
            psT2 = psend.tile([1, BS], F32, tag="psT2")
            nc.tensor.transpose(psT2, KLCN[:, 1:2], EYE)
            klrow = psT1[:, :]
            cnrow = psT2[:, :]

            # total = 0.99*(chamfer + class + 0.001*classnum) + 0.01*kl
            # chamfer = 2*(ch1+ch2); class = -(t1+t2).
            # is_equal mode: dot = t1+t2, corr = 0            -> class = -dot
            # sign mode:     dot = 2*Sci*Scp - (t1+t2), corr = 2*Sci*Scp
            #                                                  -> class = dot - corr
            dot_sc = 0.99 if MASK_ON_ACT else -0.99
            tot = singles.tile([1, BS], F32, tag="tot")
            nc.vector.tensor_add(tot, chrows[0], chrows[1])
            nc.vector.tensor_scalar(out=tot, in0=tot, scalar1=2.0 * 0.99, scalar2=None, op0=OP.mult)
            u = singles.tile([1, BS], F32, tag="u")
            nc.vector.scalar_tensor_tensor(
                out=u, in0=pstd, scalar=dot_sc, in1=tot, op0=OP.mult, op1=OP.add
            )
            nc.vector.scalar_tensor_tensor(
                out=tot, in0=CORR, scalar=-0.99, in1=u, op0=OP.mult, op1=OP.add
            )
            nc.vector.scalar_tensor_tensor(
                out=u, in0=cnrow, scalar=0.99 * 0.001, in1=tot, op0=OP.mult, op1=OP.add
            )
            nc.vector.scalar_tensor_tensor(
                out=tot, in0=klrow, scalar=0.01, in1=u, op0=OP.mult, op1=OP.add
            )
            nc.sync.dma_start(out=out.rearrange("(o s) -> o s", o=1), in_=tot)


_NC_CACHE = None


def _get_nc():
    global _NC_CACHE
    if _NC_CACHE is None:
        _NC_CACHE = _build_core_program()
    return _NC_CACHE


def _split3(v):
    import ml_dtypes
    a = v.astype(ml_dtypes.bfloat16)
    r = v - a.astype(np.float32)
    b_ = r.astype(ml_dtypes.bfloat16)
    c = (r - b_.astype(np.float32)).astype(ml_dtypes.bfloat16)
    return a, b_, c


def _build_ops(xT, yT):
    """ops[s, c, r, :]: cols 0:128 lhsT rows (x chunk c), cols 128: rhs rows (y, bf16).

    Rows 0..23: 6 bf16-split product pairs x 4 dims; rows 24..26: -0.5|y|^2 via
    ones-column lhsT; rows 27..31: zero padding (keeps the DMA a clean [128,640]).
    """
    import ml_dtypes
    bf = ml_dtypes.bfloat16
    xa, xb, xc = _split3(xT)              # [B, K, N] each
    ya, yb, yc = _split3(yT)
    w = -0.5 * (yT.astype(np.float64) ** 2).sum(axis=1)   # [B, N]
    wa, wb, wc = _split3(w.astype(np.float32))
    ops = np.zeros((B, NCH, 32, 128 + N), bf)
    pairs = [(xa, ya), (xa, yb), (xb, ya), (xb, yb), (xa, yc), (xc, ya)]
    for c in range(NCH):
        for p, (xs, ys) in enumerate(pairs):
            r = 4 * p
            ops[:, c, r : r + K, 0:128] = xs[:, :, 128 * c : 128 * c + 128]
            ops[:, c, r : r + K, 128:] = ys
        ones = np.ones((128,), bf)
        for j, ws in enumerate((wa, wb, wc)):
            ops[:, c, 24 + j, 0:128] = ones
            ops[:, c, 24 + j, 128:] = ws
    return ops


def build_in_maps(inputs):
    import ml_dtypes
    bf = ml_dtypes.bfloat16
    ki = np.ascontiguousarray(np.asarray(inputs["kine_input"], dtype=np.float32))
    kp = np.ascontiguousarray(np.asarray(inputs["kine_pred"], dtype=np.float32))
    cli = np.ascontiguousarray(np.asarray(inputs["class_input"], dtype=np.float32))
    clp = np.ascontiguousarray(np.asarray(inputs["class_pred"], dtype=np.float32))
    mu = np.ascontiguousarray(np.asarray(inputs["mu"], dtype=np.float32))
    lv = np.ascontiguousarray(np.asarray(inputs["log_var"], dtype=np.float32))

    kiT = np.ascontiguousarray(ki.transpose(0, 2, 1))
    kpT = np.ascontiguousarray(kp.transpose(0, 2, 1))
    ops1 = _build_ops(kiT, kpT)           # [B, NCH, 32, 640]
    ops2 = _build_ops(kpT, kiT)

    # 0.5|x|^2 in [128, BS*NCH] layout (col = 4s+c)
    x2i = 0.5 * (ki.astype(np.float64) ** 2).sum(-1).astype(np.float32)   # [B, N]
    x2p = 0.5 * (kp.astype(np.float64) ** 2).sum(-1).astype(np.float32)
    x2h1 = x2i.reshape(NCORES, BS, NCH, 128).transpose(0, 3, 1, 2).reshape(NCORES, 128, BS * NCH)
    x2h2 = x2p.reshape(NCORES, BS, NCH, 128).transpose(0, 3, 1, 2).reshape(NCORES, 128, BS * NCH)

    # class data, point-chunk partition layout [128, BS, NCH, D]
    cib5 = cli.reshape(NCORES, BS, NCH, 128, D).transpose(0, 3, 1, 2, 4)
    cpb5 = clp.reshape(NCORES, BS, NCH, 128, D).transpose(0, 3, 1, 2, 4)
    cib = np.ascontiguousarray(cib5.astype(bf))
    cpb = np.ascontiguousarray(cpb5.astype(bf))
    dotp = np.concatenate(
        [cpb5.reshape(NCORES, 128, BS, NCH * D), cib5.reshape(NCORES, 128, BS, NCH * D)],
        axis=3,
    )
    dotp = np.ascontiguousarray(dotp.astype(np.float32))

    # histogram layout [128(=32c+s), 128*D]
    hci = np.ascontiguousarray(
        cli.reshape(NCORES, BS, NCH, 128 * D).transpose(0, 2, 1, 3).reshape(NCORES, 128, 128 * D)
    )
    hcp = np.ascontiguousarray(
        clp.reshape(NCORES, BS, NCH, 128 * D).transpose(0, 2, 1, 3).reshape(NCORES, 128, 128 * D)
    )

    if MASK_ON_ACT:
        # dir1 embeds sum_d (sum_n cib_bf16) * (sum_m cp_f32); dir2 symmetric.
        sci_b = cib.astype(np.float64).sum(axis=(1, 3))   # [NCORES, BS, D]
        scp_b = cpb.astype(np.float64).sum(axis=(1, 3))
        sci_f = cli.reshape(NCORES, BS, N, D).astype(np.float64).sum(axis=2)
        scp_f = clp.reshape(NCORES, BS, N, D).astype(np.float64).sum(axis=2)
        corr = ((sci_b * scp_f + scp_b * sci_f).sum(-1)).astype(np.float32)[:, None, :]
    else:
        corr = np.zeros((NCORES, 1, BS), np.float32)

    ops1 = ops1.reshape(NCORES, BS, NCH, 32, 128 + N)
    ops2 = ops2.reshape(NCORES, BS, NCH, 32, 128 + N)
    mu_s = mu.reshape(NCORES, BS, L)
    lv_s = lv.reshape(NCORES, BS, L)

    in_maps = []
    for c in range(NCORES):
        in_maps.append(
            {
                "ops1": ops1[c], "ops2": ops2[c],
                "cib": cib[c], "cpb": cpb[c], "dotp": dotp[c],
                "x2h1": x2h1[c], "x2h2": x2h2[c],
                "hci": hci[c], "hcp": hcp[c],
                "mu": mu_s[c], "lv": lv_s[c],
                "corr": corr[c],
                **CONSTS,
            }
        )
    return in_maps


def kernel(**inputs):
    global LAST_RESULT
    in_maps = build_in_maps(inputs)
    nc = _get_nc()
    res = run_bass_kernel_spmd(nc, in_maps, list(range(NCORES)), trace=TRACE)
    LAST_RESULT = res
    outs = [np.asarray(res.results[c]["out"], dtype=np.float32) for c in range(NCORES)]
    return np.concatenate(outs, axis=0)


# revision 29
# speedup vs baseline: 1.1937x; 1.1937x over previous
"""Trainium2 Bass kernel for nn_LossFunc_13752485282042 (chamfer + class + KL + histogram loss).

Contract: kernel(**inputs) takes FULL unsharded numpy inputs (B=256) and returns the
full [256] f32 loss vector. Internally shards batch across 8 NeuronCores (pure data
parallel, 32 samples/core) and runs one SPMD Bass/Tile kernel.

Algorithm per sample (N=M=512 points, K=4 kine dims, D=9 classes):
  f1[n,m] = sum_k x[n,k]*y[m,k] - 0.5*|y[m]|^2        (row-tiled KC=27 bf16-split matmuls)
  min_m d = 2*relu(0.5|x_n|^2 - max_m f1); row max via DVE tensor_reduce(max).
  Class gather is realized WITHOUT index extraction: a {0,1} winner mask
  mask1[n,m] = (f1[n,m] == rowmax1[n]) (DVE is_equal, or ACT Sign with a host-side
  colsum correction), then C1[m,d] = sum_n mask1[n,m]*ci[n,d] via 16 small matmuls,
  and term1 = <C1, cp> via one DVE scalar_tensor_tensor with accum. Same transposed
  for the pred->input direction. Histogram/classnum via equality masks + selector
  matmuls; KL via ACT exp-accumulate. No GPSIMD gathers, no index relayout DMAs.
"""

import numpy as np

import concourse.bass as bass
import concourse.bacc as bacc
import concourse.mybir as mybir
from concourse.tile import TileContext
from concourse.bass_utils import run_bass_kernel_spmd

F32 = mybir.dt.float32
BF16 = mybir.dt.bfloat16
AX = mybir.AxisListType
OP = mybir.AluOpType
ACT = mybir.ActivationFunctionType

B, N, K, D, L = 256, 512, 4, 9, 32
KC = 27                   # matmul contraction rows: 6 bf16-split products x 4 dims + 3 |y|^2 rows
NCORES = 8
BS = B // NCORES          # 32 samples per core
NCH = N // 128            # 4 partition chunks per sample

# Winner-mask engine: False -> DVE is_equal ({0,1} mask, no correction);
# True -> ACT Sign(rowmax - f1) ({0 winner, +1 loser} mask, host colsum correction).
import os as _os
MASK_ON_ACT = _os.environ.get("MASK_ON_ACT", "1") == "1"
OPS_BUFS = int(_os.environ.get("OPS_BUFS", "6"))
MASK_BUFS = int(_os.environ.get("MASK_BUFS", "8"))
PS_BUFS = int(_os.environ.get("PS_BUFS", "3"))
C_BUFS = int(_os.environ.get("C_BUFS", "2"))
# move one winner-mask chunk per sample from ACT to DVE on samples where
# (s % MASK_MIX_MOD) == 1; 0 disables
MASK_MIX_MOD = int(_os.environ.get("MASK_MIX_MOD", "0"))

TRACE = False             # set by test.py to collect a profile
LAST_RESULT = None


def _build_core_program():
    nc = bacc.Bacc()

    ops1 = nc.declare_dram_parameter("ops1", [BS, NCH, 32, 128 + N], BF16, isOutput=False)
    ops2 = nc.declare_dram_parameter("ops2", [BS, NCH, 32, 128 + N], BF16, isOutput=False)
    cib = nc.declare_dram_parameter("cib", [128, BS, NCH, D], BF16, isOutput=False)
    cpb = nc.declare_dram_parameter("cpb", [128, BS, NCH, D], BF16, isOutput=False)
    dotp = nc.declare_dram_parameter("dotp", [128, BS, 2 * NCH * D], F32, isOutput=False)
    x2h1 = nc.declare_dram_parameter("x2h1", [128, BS * NCH], F32, isOutput=False)
    x2h2 = nc.declare_dram_parameter("x2h2", [128, BS * NCH], F32, isOutput=False)
    hci = nc.declare_dram_parameter("hci", [128, 128 * D], F32, isOutput=False)
    hcp = nc.declare_dram_parameter("hcp", [128, 128 * D], F32, isOutput=False)
    mu = nc.declare_dram_parameter("mu", [BS, L], F32, isOutput=False)
    lv = nc.declare_dram_parameter("lv", [BS, L], F32, isOutput=False)
    corr = nc.declare_dram_parameter("corr", [1, BS], F32, isOutput=False)
    c_eye = nc.declare_dram_parameter("c_eye", [BS, BS], F32, isOutput=False)
    c_ones = nc.declare_dram_parameter("c_ones", [128, 1], F32, isOutput=False)
    c_selp = nc.declare_dram_parameter("c_selp", [128, BS], F32, isOutput=False)
    c_seln = nc.declare_dram_parameter("c_seln", [128, BS], F32, isOutput=False)
    c_wrep = nc.declare_dram_parameter("c_wrep", [BS, D], F32, isOutput=False)
    out = nc.declare_dram_parameter("out", [BS], F32, isOutput=True)

    with TileContext(nc) as tc:
        _emit(nc, tc, ops1, ops2, cib, cpb, dotp, x2h1, x2h2, hci, hcp,
              mu, lv, corr, c_eye, c_ones, c_selp, c_seln, c_wrep, out)
    nc.finalize()
    return nc


def _host_consts():
    selp = np.zeros((128, BS), np.float32)
    for s in range(BS):
        for c in range(4):
            selp[32 * c + s, s] = 1.0
    wrep = np.ones((BS, D), np.float32)
    wrep[:, 0] = 2.0
    wrep[:, D - 1] = 100.0
    return {
        "c_eye": np.eye(BS, dtype=np.float32),
        "c_ones": np.ones((128, 1), np.float32),
        "c_selp": selp,
        "c_seln": -selp,
        "c_wrep": wrep,
    }


CONSTS = _host_consts()


def _emit(nc, tc, ops1d, ops2d, cibd, cpbd, dotpd, x2h1d, x2h2d, hcid, hcpd,
          mud, lvd, corrd, c_eye, c_ones, c_selp, c_seln, c_wrep, out):
    from contextlib import ExitStack

    ctx = ExitStack()
    with ctx:
        singles = ctx.enter_context(tc.tile_pool(name="singles", bufs=1))
        opsp = ctx.enter_context(tc.tile_pool(name="opsp", bufs=OPS_BUFS))
        maskp = ctx.enter_context(tc.tile_pool(name="maskp", bufs=MASK_BUFS))

        # ---------- tiles needed by the main loop ----------
        # class-data loads are quartered and interleaved with the first OPS
        # loads (emitted inside the loop) so sample 0's pipeline starts early
        CIB = singles.tile([128, BS, NCH, D], BF16, tag="CIB")
        CPB = singles.tile([128, BS, NCH, D], BF16, tag="CPB")
        DOTP = singles.tile([128, BS, 2 * NCH * D], F32, tag="DOTP")

        def _cls_quarter(q):
            sl = slice(8 * q, 8 * q + 8)
            nc.sync.dma_start(out=CIB[:, sl], in_=cibd[:, sl])
            nc.sync.dma_start(out=CPB[:, sl], in_=cpbd[:, sl])
            nc.sync.dma_start(out=DOTP[:, sl], in_=dotpd[:, sl])

        RMAX1 = singles.tile([128, BS * NCH], F32, tag="RMAX1")
        RMAX2 = singles.tile([128, BS * NCH], F32, tag="RMAX2")
        TD = singles.tile([128, BS], F32, tag="TD")
        DOTSCR = singles.tile([128, 2 * NCH * D], F32, tag="DOTSCR")

        ONES128 = singles.tile([128, 1], F32, tag="ONES128")
        SELP = singles.tile([128, BS], F32, tag="SELP")
        SELN = singles.tile([128, BS], F32, tag="SELN")
        WREP = singles.tile([BS, D], F32, tag="WREP")
        CORR = singles.tile([1, BS], F32, tag="CORR")
        EYE = singles.tile([BS, BS], F32, tag="EYE")
        X2H1 = singles.tile([128, BS * NCH], F32, tag="X2H1")
        X2H2 = singles.tile([128, BS * NCH], F32, tag="X2H2")
        HCI = singles.tile([128, 128 * D], F32, tag="HCI")
        HCP = singles.tile([128, 128 * D], F32, tag="HCP")
        KLCN = singles.tile([BS, 2], F32, tag="KLCN")

        def _emit_side_loads_a():
            nc.sync.dma_start(out=ONES128, in_=c_ones[:])
            nc.sync.dma_start(out=SELP, in_=c_selp[:])
            nc.sync.dma_start(out=SELN, in_=c_seln[:])
            nc.sync.dma_start(out=WREP, in_=c_wrep[:])
            nc.sync.dma_start(out=CORR, in_=corrd[:])
            nc.sync.dma_start(out=EYE, in_=c_eye[:])
            nc.sync.dma_start(out=X2H1, in_=x2h1d[:])
            nc.sync.dma_start(out=X2H2, in_=x2h2d[:])

        def _emit_side_loads_b():
            nc.sync.dma_start(out=HCI, in_=hcid[:])
            nc.sync.dma_start(out=HCP, in_=hcpd[:])

        def _emit_side_loads():
            # KL -> KLCN[:, 0]
            smu = singles.tile([BS, L], F32, tag="smu")
            slv = singles.tile([BS, L], F32, tag="slv")
            nc.sync.dma_start(out=smu, in_=mud[:])
            nc.sync.dma_start(out=slv, in_=lvd[:])
            sexp = singles.tile([BS, L], F32, tag="sexp")
            sumexp = singles.tile([BS, 1], F32, tag="sumexp")
            nc.scalar.activation(sexp, slv, ACT.Exp, accum_out=sumexp)
            smu2 = singles.tile([BS, L], F32, tag="smu2")
            summu2 = singles.tile([BS, 1], F32, tag="summu2")
            nc.vector.scalar_tensor_tensor(
                out=smu2, in0=smu, scalar=1.0, in1=smu,
                op0=OP.mult, op1=OP.mult, accum_out=summu2,
            )
            sumlv = singles.tile([BS, 1], F32, tag="sumlv")
            nc.vector.reduce_sum(sumlv, slv, axis=AX.X)
            klt = KLCN[:, 0:1]
            nc.vector.tensor_sub(klt, sumlv, summu2)
            nc.vector.tensor_sub(klt, klt, sumexp)
            nc.vector.tensor_scalar(out=klt, in0=klt, scalar1=float(L), scalar2=None, op0=OP.add)
            nc.vector.tensor_scalar(out=klt, in0=klt, scalar1=-0.5, scalar2=None, op0=OP.mult)

        def _emit_hist(psC):
            # histogram counts (equality mask on the otherwise-idle GpSimd),
            # then classnum -> KLCN[:, 1]
            cnts = []
            for name, h in (("i", HCI), ("p", HCP)):
                rmx = singles.tile([128, 128], F32, tag=f"rmx{name}")
                nc.vector.reduce_max(rmx, h.rearrange("p (n d) -> p n d", d=D), axis=AX.X)
                oh = singles.tile([128, 128 * D], F32, tag=f"oh{name}")
                nc.vector.tensor_tensor(
                    out=oh.rearrange("p (d n) -> p n d", d=D),
                    in0=h.rearrange("p (n d) -> p n d", d=D),
                    in1=rmx.to_broadcast([128, 128, D]),
                    op=OP.is_equal,
                )
                cnt = singles.tile([128, D], F32, tag=f"cnt{name}")
                nc.vector.reduce_sum(cnt, oh.rearrange("p (d n) -> p d n", d=D), axis=AX.X)
                cnts.append(cnt)
            psh = psC.tile([128, 2 * NCH * D], F32, tag="C")
            nc.tensor.matmul(psh[0:BS, 0:D], SELP, cnts[0], start=True, stop=False)
            nc.tensor.matmul(psh[0:BS, 0:D], SELN, cnts[1], start=False, stop=True)
            habs = singles.tile([BS, D], F32, tag="habs")
            nc.scalar.activation(habs, psh[0:BS, 0:D], ACT.Abs)
            hw_ = singles.tile([BS, D], F32, tag="hw_")
            nc.vector.tensor_mul(hw_, habs, WREP)
            nc.vector.reduce_sum(KLCN[:, 1:2], hw_, axis=AX.X)

        # ---------- main per-sample loop ----------
        with tc.tile_pool(name="psPS", bufs=PS_BUFS, space="PSUM") as psPS, \
             tc.tile_pool(name="psC", bufs=C_BUFS, space="PSUM") as psC:
            for s in range(BS):
                C = psC.tile([128, 2 * NCH * D], F32, tag="C")
                for di, (opsd, RMAXd, CLSR) in enumerate(
                    [(ops1d, RMAX1, CIB), (ops2d, RMAX2, CPB)]
                ):
                    OPS = opsp.tile([128, 128 + N], BF16, tag="OPS")
                    nc.sync.dma_start(
                        out=OPS, in_=opsd[s].rearrange("c r x -> (c r) x")
                    )
                    if s == 0 and di == 0:
                        _cls_quarter(0)
                    elif s == 1 and di == 0:
                        _cls_quarter(1)
                    elif s == 2 and di == 0:
                        _cls_quarter(2)
                        _cls_quarter(3)
                    masks = []
                    for h in range(2):
                        PS = psPS.tile([128, 2, N], F32, tag="PS")
                        for j in range(2):
                            c = 2 * h + j
                            base = 32 * c
                            nc.tensor.matmul(
                                PS[:, j],
                                OPS[base : base + KC, 0:128],
                                OPS[base : base + KC, 128:],
                                start=True, stop=True,
                                tile_position=(base, 0),
                            )
                        rms = RMAXd[:, NCH * s + 2 * h : NCH * s + 2 * h + 2]
                        nc.vector.reduce_max(rms, PS, axis=AX.X)
                        for j in range(2):
                            c = 2 * h + j
                            M = maskp.tile([128, N], BF16, tag="M")
                            on_dve = (
                                MASK_MIX_MOD
                                and s % MASK_MIX_MOD == 1
                                and di == 1 and h == 1 and j == 1
                            )
                            if MASK_ON_ACT and not on_dve:
                                # {0 at winner, +1 at losers}
                                nc.scalar.activation(
                                    M, PS[:, j], ACT.Sign,
                                    bias=rms[:, j : j + 1], scale=-1.0,
                                )
                            elif MASK_ON_ACT:
                                # DVE equivalent of the Sign mask: rmax - PS,
                                # clamped into {0 loser, ... } -- use is_gt:
                                # (rmax > PS) -> 1 at losers, 0 at winner
                                nc.vector.tensor_tensor(
                                    out=M,
                                    in0=rms[:, j : j + 1].to_broadcast([128, N]),
                                    in1=PS[:, j],
                                    op=OP.is_gt,
                                )
                            else:
                                # {1 at winner, 0 at losers}
                                nc.vector.tensor_tensor(
                                    out=M, in0=PS[:, j],
                                    in1=rms[:, j : j + 1].to_broadcast([128, N]),
                                    op=OP.is_equal,
                                )
                            masks.append(M)
                    # PSUM zeroing is bank-granular at group start, so each
                    # (di, mc) accumulation group must run back-to-back.
                    for mc in range(NCH):
                        for c in range(NCH):
                            nc.tensor.matmul(
                                C[:, D * (NCH * di + mc) : D * (NCH * di + mc) + D],
                                masks[c][:, 128 * mc : 128 * mc + 128],
                                CLSR[:, s, c, :],
                                start=(c == 0), stop=(c == NCH - 1),
                            )
                nc.vector.scalar_tensor_tensor(
                    out=DOTSCR, in0=C, scalar=1.0, in1=DOTP[:, s],
                    op0=OP.mult, op1=OP.mult,
                    accum_out=TD[:, s : s + 1],
                )
                if s == 3:
                    _emit_side_loads_a()
                    _emit_side_loads_b()
                    _emit_side_loads()
                elif s == 26:
                    _emit_hist(psC)

        # ---------- final assembly ----------
        with tc.tile_pool(name="psend", bufs=1, space="PSUM") as psend:
            # chamfer: relu(0.5|x|^2 - rowmax), summed over points
            chrows = []
            for X2H, RMAX, nm in ((X2H1, RMAX1, "1"), (X2H2, RMAX2, "2")):
                T = singles.tile([128, BS * NCH], F32, tag=f"T{nm}")
                nc.vector.scalar_tensor_tensor(
                    out=T, in0=RMAX, scalar=-1.0, in1=X2H,
                    op0=OP.mult, op1=OP.add,
                )
                R = singles.tile([128, BS * NCH], F32, tag=f"R{nm}")
                nc.vector.tensor_scalar(out=R, in0=T, scalar1=0.0, scalar2=None, op0=OP.max)
                pse = psend.tile([1, BS * NCH], F32, tag=f"pse{nm}")
                nc.tensor.matmul(pse, ONES128, R, start=True, stop=True)
                ch = singles.tile([1, BS], F32, tag=f"ch{nm}")
                nc.vector.reduce_sum(ch, pse.rearrange("o (s c) -> o s c", c=NCH), axis=AX.X)
                chrows.append(ch)

            # class-gather dot: partition-sum of TD
            pstd = psend.tile([1, BS], F32, tag="pstd")
            nc.tensor.matmul(pstd, ONES128, TD, start=True, stop=True)

            # [BS,1] -> [1,BS] via PE transpose (PSUM reads must start at partition 0)
            psT1 = psend.tile([1, BS], F32, tag="psT1")
            nc.tensor.transpose(psT1, KLCN[:, 0:1], EYE)
            psT2 = psend.tile([1, BS], F32, tag="psT2")
            nc.tensor.transpose(psT2, KLCN[:, 1:2], EYE)
            klrow = psT1[:, :]
            cnrow = psT2[:, :]

            # total = 0.99*(chamfer + class + 0.001*classnum) + 0.01*kl
            # chamfer = 2*(ch1+ch2); class = -(t1+t2).
            # is_equal mode: dot = t1+t2, corr = 0            -> class = -dot
            # sign mode:     dot = 2*Sci*Scp - (t1+t2), corr = 2*Sci*Scp
            #                                                  -> class = dot - corr
            dot_sc = 0.99 if MASK_ON_ACT else -0.99
            tot = singles.tile([1, BS], F32, tag="tot")
            nc.vector.tensor_add(tot, chrows[0], chrows[1])
            nc.vector.tensor_scalar(out=tot, in0=tot, scalar1=2.0 * 0.99, scalar2=None, op0=OP.mult)
            u = singles.tile([1, BS], F32, tag="u")
            nc.vector.scalar_tensor_tensor(
                out=u, in0=pstd, scalar=dot_sc, in1=tot, op0=OP.mult, op1=OP.add
            )
            nc.vector.scalar_tensor_tensor(
                out=tot, in0=CORR, scalar=-0.99, in1=u, op0=OP.mult, op1=OP.add
            )
            nc.vector.scalar_tensor_tensor(
                out=u, in0=cnrow, scalar=0.99 * 0.001, in1=tot, op0=OP.mult, op1=OP.add
            )
            nc.vector.scalar_tensor_tensor(
                out=tot, in0=klrow, scalar=0.01, in1=u, op0=OP.mult, op1=OP.add
            )
            nc.sync.dma_start(out=out.rearrange("(o s) -> o s", o=1), in_=tot)


_NC_CACHE = None


def _get_nc():
    global _NC_CACHE
    if _NC_CACHE is None:
        _NC_CACHE = _build_core_program()
    return _NC_CACHE


def _split3(v):
    import ml_dtypes
    a = v.astype(ml_dtypes.bfloat16)
    r = v - a.astype(np.float32)
    b_ = r.astype(ml_dtypes.bfloat16)
    c = (r - b_.astype(np.float32)).astype(ml_dtypes.bfloat16)
    return a, b_, c


def _build_ops(xT, yT):
    """ops[s, c, r, :]: cols 0:128 lhsT rows (x chunk c), cols 128: rhs rows (y, bf16).

    Rows 0..23: 6 bf16-split product pairs x 4 dims; rows 24..26: -0.5|y|^2 via
    ones-column lhsT; rows 27..31: zero padding (keeps the DMA a clean [128,640]).
    """
    import ml_dtypes
    bf = ml_dtypes.bfloat16
    xa, xb, xc = _split3(xT)              # [B, K, N] each
    ya, yb, yc = _split3(yT)
    w = -0.5 * (yT.astype(np.float64) ** 2).sum(axis=1)   # [B, N]
    wa, wb, wc = _split3(w.astype(np.float32))
    ops = np.zeros((B, NCH, 32, 128 + N), bf)
    pairs = [(xa, ya), (xa, yb), (xb, ya), (xb, yb), (xa, yc), (xc, ya)]
    for c in range(NCH):
        for p, (xs, ys) in enumerate(pairs):
            r = 4 * p
            ops[:, c, r : r + K, 0:128] = xs[:, :, 128 * c : 128 * c + 128]
            ops[:, c, r : r + K, 128:] = ys
        ones = np.ones((128,), bf)
        for j, ws in enumerate((wa, wb, wc)):
            ops[:, c, 24 + j, 0:128] = ones
            ops[:, c, 24 + j, 128:] = ws
    return ops


def build_in_maps(inputs):
    import ml_dtypes
    bf = ml_dtypes.bfloat16
    ki = np.ascontiguousarray(np.asarray(inputs["kine_input"], dtype=np.float32))
    kp = np.ascontiguousarray(np.asarray(inputs["kine_pred"], dtype=np.float32))
    cli = np.ascontiguousarray(np.asarray(inputs["class_input"], dtype=np.float32))
    clp = np.ascontiguousarray(np.asarray(inputs["class_pred"], dtype=np.float32))
    mu = np.ascontiguousarray(np.asarray(inputs["mu"], dtype=np.float32))
    lv = np.ascontiguousarray(np.asarray(inputs["log_var"], dtype=np.float32))

    kiT = np.ascontiguousarray(ki.transpose(0, 2, 1))
    kpT = np.ascontiguousarray(kp.transpose(0, 2, 1))
    ops1 = _build_ops(kiT, kpT)           # [B, NCH, 32, 640]
    ops2 = _build_ops(kpT, kiT)

    # 0.5|x|^2 in [128, BS*NCH] layout (col = 4s+c)
    x2i = 0.5 * (ki.astype(np.float64) ** 2).sum(-1).astype(np.float32)   # [B, N]
    x2p = 0.5 * (kp.astype(np.float64) ** 2).sum(-1).astype(np.float32)
    x2h1 = x2i.reshape(NCORES, BS, NCH, 128).transpose(0, 3, 1, 2).reshape(NCORES, 128, BS * NCH)
    x2h2 = x2p.reshape(NCORES, BS, NCH, 128).transpose(0, 3, 1, 2).reshape(NCORES, 128, BS * NCH)

    # class data, point-chunk partition layout [128, BS, NCH, D]
    cib5 = cli.reshape(NCORES, BS, NCH, 128, D).transpose(0, 3, 1, 2, 4)
    cpb5 = clp.reshape(NCORES, BS, NCH, 128, D).transpose(0, 3, 1, 2, 4)
    cib = np.ascontiguousarray(cib5.astype(bf))
    cpb = np.ascontiguousarray(cpb5.astype(bf))
    dotp = np.concatenate(
        [cpb5.reshape(NCORES, 128, BS, NCH * D), cib5.reshape(NCORES, 128, BS, NCH * D)],
        axis=3,
    )
    dotp = np.ascontiguousarray(dotp.astype(np.float32))

    # histogram layout [128(=32c+s), 128*D]
    hci = np.ascontiguousarray(
        cli.reshape(NCORES, BS, NCH, 128 * D).transpose(0, 2, 1, 3).reshape(NCORES, 128, 128 * D)
    )
    hcp = np.ascontiguousarray(
        clp.reshape(NCORES, BS, NCH, 128 * D).transpose(0, 2, 1, 3).reshape(NCORES, 128, 128 * D)
    )

    if MASK_ON_ACT:
        # dir1 embeds sum_d (sum_n cib_bf16) * (sum_m cp_f32); dir2 symmetric.
        sci_b = cib.astype(np.float64).sum(axis=(1, 3))   # [NCORES, BS, D]
        scp_b = cpb.astype(np.float64).sum(axis=(1, 3))
        sci_f = cli.reshape(NCORES, BS, N, D).astype(np.float64).sum(axis=2)
        scp_f = clp.reshape(NCORES, BS, N, D).astype(np.float64).sum(axis=2)
        corr = ((sci_b * scp_f + scp_b * sci_f).sum(-1)).astype(np.float32)[:, None, :]
    else:
        corr = np.zeros((NCORES, 1, BS), np.float32)

    ops1 = ops1.reshape(NCORES, BS, NCH, 32, 128 + N)
    ops2 = ops2.reshape(NCORES, BS, NCH, 32, 128 + N)
    mu_s = mu.reshape(NCORES, BS, L)
    lv_s = lv.reshape(NCORES, BS, L)

    in_maps = []
    for c in range(NCORES):
        in_maps.append(
            {
                "ops1": ops1[c], "ops2": ops2[c],
                "cib": cib[c], "cpb": cpb[c], "dotp": dotp[c],
                "x2h1": x2h1[c], "x2h2": x2h2[c],
                "hci": hci[c], "hcp": hcp[c],
                "mu": mu_s[c], "lv": lv_s[c],
                "corr": corr[c],
                **CONSTS,
            }
        )
    return in_maps


def kernel(**inputs):
    global LAST_RESULT
    in_maps = build_in_maps(inputs)
    nc = _get_nc()
    res = run_bass_kernel_spmd(nc, in_maps, list(range(NCORES)), trace=TRACE)
    LAST_RESULT = res
    outs = [np.asarray(res.results[c]["out"], dtype=np.float32) for c in range(NCORES)]
    return np.concatenate(outs, axis=0)
